# revision 5
# baseline (speedup 1.0000x reference)
"""Trainium2 Bass kernel for the CGC multi-task MoE routing module.

Math: the reference computes, per task t:
    expert outputs  E[t,e] = x @ W[t,e] + b[t,e]          (ES specific + EC common)
    gate logits     L[t]   = concat_e(E[t,e]) @ Wg[t] + bg[t]
    weights         p      = softmax(L[t])
    feature         F[t]   = sum_e p_e * E[t,e]
    out[t]          = F[t] @ Wt[t] + bt[t]                # scalar per sample

Both L[t] and the per-expert scalars s[t,e] = E[t,e] @ Wt[t] are linear in x,
so everything folds into one skinny matmul z = x @ A + d with
A: [I, 24] (per task: 6 logit cols + 6 scalar cols), followed by a per-sample
6-way softmax-weighted average:
    out[t,b] = sum_e exp(L_e) * s_e / sum_e exp(L_e)      (bt folded into s).

The kernel is memory-bound on streaming x. Device traffic is halved by
shipping x (and A) as bfloat16 — quantization rel-err ~1.7e-3, well inside
the 2e-2 gate — with fp32 PSUM accumulation and an fp32 epilogue.

Device kernel (SPMD over 8 cores, batch-sharded):
  - host packs each x shard as bf16 [NQ, 128, NCHUNK, QW]: per (quad q,
    partition p) a contiguous 16 KB run covering all 4 I-chunks, so each
    quad loads with 128 large descriptors.
  - all quad loads are issued up front (8.4 MB bf16 fits SBUF easily).
  - per quad: 4 bands x 4 accumulating bf16 matmuls (lhsT = A chunk
    [128, 32], rhs = x [128, 512]) -> zT [32, GW] in PSUM; ScalarE copies
    PSUM->SBUF adding the per-partition bias d; VectorE 32x32
    block-transpose flips to a per-sample layout.
  - epilogue per quad: Exp on ScalarE, group-reductions, multiply and
    reciprocal on VectorE, results accumulated in one SBUF tile.
  - single DMA of the [128, NQ*NB*T] result tile at the end.
"""

import os

import numpy as np

B, I, H = 65536, 512, 128
T, ES, EC = 2, 2, 4
ETOT = ES + EC

N_CORES = 8
BS = B // N_CORES  # samples per core
M = 32  # folded output channels, padded 24 -> 32 for the 32x32 transpose
GW = 512  # samples per band (one PSUM bank per band)
QW = 4 * GW  # samples per quad (4 bands stacked on the 128 partitions)
NQ = BS // QW
NCHUNK = I // 128
NB = GW // 32  # 32-sample blocks per band


def _fold(inputs):
    """Fold all weights into A [I, M] and bias d [M, 1] (float32).

    Channel layout per task t (base 12*t): 0:6 gate logits, 6:12 per-expert
    scalars (bt folded in, valid since softmax weights sum to 1).
    """
    w64 = lambda k: np.asarray(inputs[k], np.float64)
    Wc, bc, Ws, bs = w64("Wc"), w64("bc"), w64("Ws"), w64("bs")
    Wg, bg, Wt, bt = w64("Wg"), w64("bg"), w64("Wt"), w64("bt")

    A = np.zeros((I, M))
    d = np.zeros(M)
    for t in range(T):
        W_all = np.concatenate(
            [Ws[t, e] for e in range(ES)] + [Wc[e] for e in range(EC)], axis=1
        )  # [I, ETOT*H]
        b_all = np.concatenate(
            [bs[t, e] for e in range(ES)] + [bc[e] for e in range(EC)]
        )  # [ETOT*H]
        A[:, 12 * t : 12 * t + 6] = W_all @ Wg[t]
        d[12 * t : 12 * t + 6] = b_all @ Wg[t] + bg[t]
        A[:, 12 * t + 6 : 12 * t + 12] = (
            W_all.reshape(I, ETOT, H) * Wt[t, :, 0][None, None, :]
        ).sum(-1)
        d[12 * t + 6 : 12 * t + 12] = (
            b_all.reshape(ETOT, H) * Wt[t, :, 0][None, :]
        ).sum(-1) + bt[t, 0]
    return A.astype(np.float32), d.reshape(M, 1).astype(np.float32)


def _build_program():
    import concourse.bacc as bacc
    import concourse.mybir as mybir
    from concourse.tile import TileContext

    f32 = mybir.dt.float32
    bf16 = mybir.dt.bfloat16

    nc = bacc.Bacc("TRN2", target_bir_lowering=False, debug=False, num_devices=N_CORES)
    # xq[g, p, c, w]: I-index = c*128 + p, sample = g*GW + w  (g = band)
    xq_ext = nc.declare_dram_parameter("xq", [4 * NQ, 128, NCHUNK, GW], bf16, isOutput=False)
    A_ext = nc.declare_dram_parameter("A", [I, M], bf16, isOutput=False)
    d_ext = nc.declare_dram_parameter("d", [M, 1], f32, isOutput=False)
    # out[p, q, blk, t]: sample s = q*QW + (p//32)*GW + 32*blk + p%32, task t
    out_ext = nc.declare_dram_parameter("out", [128, NQ, NB, T], f32, isOutput=True)

    with TileContext(nc) as tc:
        with (
            tc.tile_pool(name="consts", bufs=1) as cpool,
            tc.tile_pool(name="xin", bufs=NQ) as xpool,
            tc.tile_pool(name="zt", bufs=3) as ztpool,
            tc.tile_pool(name="zq", bufs=3) as zqpool,
            tc.tile_pool(name="epi", bufs=4) as epool,
            tc.tile_pool(name="psum", bufs=2, space="PSUM") as ppool,
        ):
            A_sb = cpool.tile([128, NCHUNK, M], bf16)
            nc.sync.dma_start(
                out=A_sb[:], in_=A_ext[:, :].rearrange("(c p) m -> p c m", p=128)
            )
            d_sb = cpool.tile([M, 1], f32)
            nc.sync.dma_start(out=d_sb[:], in_=d_ext[:, :])
            res_all = cpool.tile([128, NQ, NB, T], f32)

            # prefetch every band immediately: 128 x 4KB descriptors per
            # band, 8.4 MB total in SBUF; band granularity lets the first
            # matmul start ~1.3us into the stream instead of waiting for a
            # full 2MB quad
            x_tiles = []
            for g in range(4 * NQ):
                x_sb = xpool.tile([128, NCHUNK, GW], bf16, name=f"x_{g}", tag="x")
                nc.sync.dma_start(out=x_sb[:], in_=xq_ext[g, :, :, :])
                x_tiles.append(x_sb)

            from collections import deque

            pending = deque()
            for q in range(NQ):
                # z^T for the 4 bands, stacked on partitions by the bias-add
                zT_sb = ztpool.tile([128, GW], f32, name=f"zTsb_{q}", tag="zTsb")
                for j in range(4):
                    x_sb = x_tiles[4 * q + j]
                    ps = ppool.tile([M, GW], f32, name=f"ps_{q}_{j}", tag=f"ps{j}")
                    for c in range(NCHUNK):
                        nc.tensor.matmul(
                            ps[:, :],
                            A_sb[:, c, :],
                            x_sb[:, c, :],
                            start=(c == 0),
                            stop=(c == NCHUNK - 1),
                        )
                    # PSUM -> SBUF band write with per-partition bias add
                    nc.scalar.add(zT_sb[32 * j : 32 * j + 32, :], ps[:, :], d_sb[:])

                # 32x32 block transpose -> sample-major within each band
                Zq = zqpool.tile([128, GW], f32, name=f"Z_{q}", tag="Z")
                nc.vector.transpose(Zq[:], zT_sb[:])

                def emit_epilogue(q=q, Zq=Zq):
                    # softmax-weighted average, batched over the quad
                    Zb = Zq[:].rearrange("p (blk c) -> p blk c", c=32)
                    zt4 = Zb[:, :, 0:24].rearrange("p blk (t c) -> p blk t c", c=12)
                    lg = zt4[:, :, :, 0:6]
                    sc = zt4[:, :, :, 6:12]

                    ssum = epool.tile([128, NB, T], f32, name=f"ssum_{q}", tag="ssum")
                    num = epool.tile([128, NB, T], f32, name=f"num_{q}", tag="num")
                    rinv = epool.tile([128, NB, T], f32, name=f"rinv_{q}", tag="rinv")

                    nc.scalar.activation(lg, lg, mybir.ActivationFunctionType.Exp)
                    nc.vector.tensor_reduce(
                        ssum[:], lg, axis=mybir.AxisListType.X, op=mybir.AluOpType.add
                    )
                    nc.vector.tensor_mul(lg, lg, sc)  # exp * s, clobbers exp
                    nc.vector.tensor_reduce(
                        num[:], lg, axis=mybir.AxisListType.X, op=mybir.AluOpType.add
                    )
                    nc.vector.reciprocal(rinv[:], ssum[:])
                    nc.vector.tensor_mul(res_all[:, q, :, :], num[:], rinv[:])

                # defer one quad so epilogue ops don't head-of-line-block the
                # next quad's bias-adds (in-order engine queues)
                pending.append(emit_epilogue)
                if len(pending) > 1:
                    pending.popleft()()
            while pending:
                pending.popleft()()

            nc.sync.dma_start(out=out_ext[:, :, :, :], in_=res_all[:])

    nc.compile()
    return nc


_PROGRAM = None


def _ensure_ntff_hook():
    """Provide antenv.axon_hooks if the image lacks it (NTFF profiling)."""
    try:
        import antenv.axon_hooks  # noqa: F401

        return
    except ImportError:
        pass
    import contextlib
    import ctypes
    import sys
    import types

    import antenv

    mod = types.ModuleType("antenv.axon_hooks")
    holder = {"hook": None}
    mod.set_axon_ntff_profile_hook = lambda h: holder.__setitem__("hook", h)
    mod.get_axon_ntff_profile_hook = lambda: holder["hook"]
    sys.modules["antenv.axon_hooks"] = mod
    antenv.axon_hooks = mod

    so_path = "/opt/axon/libaxon_pjrt.so"
    try:
        lib = ctypes.CDLL(so_path)
    except OSError:
        return
    if not hasattr(lib, "axon_start_nrt_profile"):
        return
    lib.axon_start_nrt_profile.argtypes = [
        ctypes.POINTER(ctypes.c_int64),
        ctypes.c_size_t,
    ]
    lib.axon_start_nrt_profile.restype = ctypes.c_int64
    lib.axon_stop_nrt_profile.argtypes = [ctypes.c_char_p]
    lib.axon_stop_nrt_profile.restype = ctypes.c_int64

    @contextlib.contextmanager
    def _hook(output_dir, device_ids):
        import jax

        jax.devices()
        if device_ids:
            ids = (ctypes.c_int64 * len(device_ids))(*device_ids)
            rc = lib.axon_start_nrt_profile(ids, len(device_ids))
        else:
            rc = lib.axon_start_nrt_profile(None, 0)
        if rc != 0:
            raise RuntimeError(f"axon_start_nrt_profile rc={rc}")
        try:
            yield
        finally:
            n = lib.axon_stop_nrt_profile(str(output_dir).encode())
            print(f"ntff profile: {n} file(s) written to {output_dir}")

    mod.set_axon_ntff_profile_hook(_hook)


def _run(inputs, trace=False):
    global _PROGRAM
    import ml_dtypes

    import concourse.bass_utils as bass_utils

    if trace:
        _ensure_ntff_hook()
        # keep trace artifacts local; no bucket in this sandbox
        bass_utils.upload_artifacts = lambda tmpdir: "local://" + tmpdir

    A, d = _fold(inputs)
    A16 = A.astype(ml_dtypes.bfloat16)
    x = np.asarray(inputs["x"], np.float32)
    in_maps = []
    for i in range(N_CORES):
        # [I, BS] -> [c, p, g, w] -> [g, p, c, w], bf16 contiguous
        xT = x[i * BS : (i + 1) * BS].T.reshape(NCHUNK, 128, 4 * NQ, GW)
        xq = np.ascontiguousarray(xT.transpose(2, 1, 0, 3)).astype(ml_dtypes.bfloat16)
        in_maps.append({"xq": xq, "A": A16, "d": d})

    if _PROGRAM is None:
        _PROGRAM = _build_program()

    kres = bass_utils.run_bass_kernel_spmd(
        _PROGRAM, in_maps, core_ids=list(range(N_CORES)), trace=trace
    )

    parts = []
    for i in range(N_CORES):
        o = np.asarray(kres.results[i]["out"])  # [128, NQ, NB, T]
        # s = q*QW + j*GW + 32*blk + r with p = 32*j + r
        o = o.reshape(4, 32, NQ, NB, T)  # j, r, q, blk, t
        parts.append(o.transpose(4, 2, 0, 3, 1).reshape(T, BS))
    full = np.concatenate(parts, axis=1)[:, :, None].astype(np.float32)
    return full, kres


def kernel(**inputs):
    out, _ = _run(inputs, trace=bool(int(os.environ.get("KERNEL_TRACE", "0"))))
    return out


# revision 8
# speedup vs baseline: 1.0586x; 1.0586x over previous
"""Trainium2 Bass kernel for the CGC multi-task MoE routing module.

Math: the reference computes, per task t:
    expert outputs  E[t,e] = x @ W[t,e] + b[t,e]          (ES specific + EC common)
    gate logits     L[t]   = concat_e(E[t,e]) @ Wg[t] + bg[t]
    weights         p      = softmax(L[t])
    feature         F[t]   = sum_e p_e * E[t,e]
    out[t]          = F[t] @ Wt[t] + bt[t]                # scalar per sample

Both L[t] and the per-expert scalars s[t,e] = E[t,e] @ Wt[t] are linear in x,
so everything folds into one skinny matmul z = x @ A + d with
A: [I, 24] (per task: 6 logit cols + 6 scalar cols), followed by a per-sample
6-way softmax-weighted average:
    out[t,b] = sum_e exp(L_e) * s_e / sum_e exp(L_e)      (bt folded into s).

The kernel is memory-bound on streaming x. Device traffic is halved by
shipping x (and A) as bfloat16 — quantization rel-err ~1.7e-3, well inside
the 2e-2 gate — with fp32 PSUM accumulation and an fp32 epilogue.

Device kernel (SPMD over 8 cores, batch-sharded):
  - host packs each x shard as bf16 [NQ, 128, NCHUNK, QW]: per (quad q,
    partition p) a contiguous 16 KB run covering all 4 I-chunks, so each
    quad loads with 128 large descriptors.
  - all quad loads are issued up front (8.4 MB bf16 fits SBUF easily).
  - per quad: 4 bands x 4 accumulating bf16 matmuls (lhsT = A chunk
    [128, 32], rhs = x [128, 512]) -> zT [32, GW] in PSUM; ScalarE copies
    PSUM->SBUF adding the per-partition bias d; VectorE 32x32
    block-transpose flips to a per-sample layout.
  - epilogue per quad: Exp on ScalarE, group-reductions, multiply and
    reciprocal on VectorE, results accumulated in one SBUF tile.
  - single DMA of the [128, NQ*NB*T] result tile at the end.
"""

import os

import numpy as np

B, I, H = 65536, 512, 128
T, ES, EC = 2, 2, 4
ETOT = ES + EC

N_CORES = 8
BS = B // N_CORES  # samples per core
M = 32  # folded output channels, padded 24 -> 32 for the 32x32 transpose
GW = 512  # samples per band (one PSUM bank per band)
QW = 4 * GW  # samples per quad (4 bands stacked on the 128 partitions)
NQ = BS // QW
NCHUNK = I // 128
NB = GW // 32  # 32-sample blocks per band


def _fold(inputs):
    """Fold all weights into A [I, M] and bias d [M, 1] (float32).

    Channel layout per task t (base 12*t): 0:6 gate logits, 6:12 per-expert
    scalars (bt folded in, valid since softmax weights sum to 1).
    """
    w64 = lambda k: np.asarray(inputs[k], np.float64)
    Wc, bc, Ws, bs = w64("Wc"), w64("bc"), w64("Ws"), w64("bs")
    Wg, bg, Wt, bt = w64("Wg"), w64("bg"), w64("Wt"), w64("bt")

    A = np.zeros((I, M))
    d = np.zeros(M)
    for t in range(T):
        W_all = np.concatenate(
            [Ws[t, e] for e in range(ES)] + [Wc[e] for e in range(EC)], axis=1
        )  # [I, ETOT*H]
        b_all = np.concatenate(
            [bs[t, e] for e in range(ES)] + [bc[e] for e in range(EC)]
        )  # [ETOT*H]
        A[:, 12 * t : 12 * t + 6] = W_all @ Wg[t]
        d[12 * t : 12 * t + 6] = b_all @ Wg[t] + bg[t]
        A[:, 12 * t + 6 : 12 * t + 12] = (
            W_all.reshape(I, ETOT, H) * Wt[t, :, 0][None, None, :]
        ).sum(-1)
        d[12 * t + 6 : 12 * t + 12] = (
            b_all.reshape(ETOT, H) * Wt[t, :, 0][None, :]
        ).sum(-1) + bt[t, 0]
    return A.astype(np.float32), d.reshape(M, 1).astype(np.float32)


def _build_program():
    import concourse.bacc as bacc
    import concourse.mybir as mybir
    from concourse.tile import TileContext

    f32 = mybir.dt.float32
    bf16 = mybir.dt.bfloat16

    nc = bacc.Bacc("TRN2", target_bir_lowering=False, debug=False, num_devices=N_CORES)
    # xq[g, p, c, w]: I-index = c*128 + p, sample = g*GW + w  (g = band)
    xq_ext = nc.declare_dram_parameter("xq", [4 * NQ, 128, NCHUNK, GW], bf16, isOutput=False)
    A_ext = nc.declare_dram_parameter("A", [I, M], bf16, isOutput=False)
    d_ext = nc.declare_dram_parameter("d", [M, 1], f32, isOutput=False)
    # out[p, q, blk, t]: sample s = q*QW + (p//32)*GW + 32*blk + p%32, task t
    out_ext = nc.declare_dram_parameter("out", [128, NQ, NB, T], f32, isOutput=True)

    with TileContext(nc) as tc:
        with (
            tc.tile_pool(name="consts", bufs=1) as cpool,
            tc.tile_pool(name="xin", bufs=1) as xpool,
            tc.tile_pool(name="zt", bufs=3) as ztpool,
            tc.tile_pool(name="zq", bufs=3) as zqpool,
            tc.tile_pool(name="epi", bufs=4) as epool,
            tc.tile_pool(name="psum", bufs=2, space="PSUM") as ppool,
        ):
            A_sb = cpool.tile([128, NCHUNK, M], bf16)
            nc.sync.dma_start(
                out=A_sb[:], in_=A_ext[:, :].rearrange("(c p) m -> p c m", p=128)
            )
            d_sb = cpool.tile([M, 1], f32)
            nc.sync.dma_start(out=d_sb[:], in_=d_ext[:, :])
            res_all = cpool.tile([128, NQ, NB, T], f32)

            # prefetch all of x up front (8.4 MB, fits SBUF).  Load sizes are
            # graded: single bands at the start so the first matmul fires
            # ~0.7us into the stream, big blocks in the middle (each
            # dma_start costs ~0.6us serial descriptor-gen on Sync plus
            # fixed semaphore overhead), single bands at the end so the
            # final quad's compute isn't stuck behind a 2MB transfer.
            load_bands = [1, 1, 2, 4, 4, 2, 1, 1]
            x_tiles = []  # per band g: (tile, band offset within tile)
            g0 = 0
            for li, nb in enumerate(load_bands):
                x_sb = xpool.tile([128, nb, NCHUNK, GW], bf16, name=f"x_{li}", tag=f"x{li}")
                nc.sync.dma_start(
                    out=x_sb[:],
                    in_=xq_ext[g0 : g0 + nb, :, :, :].rearrange("g p c w -> p g c w"),
                )
                for b in range(nb):
                    x_tiles.append((x_sb, b))
                g0 += nb

            from collections import deque

            pending = deque()
            for q in range(NQ):
                # z^T for the 4 bands, stacked on partitions by the bias-add
                zT_sb = ztpool.tile([128, GW], f32, name=f"zTsb_{q}", tag="zTsb")
                for j in range(4):
                    x_sb, b = x_tiles[4 * q + j]
                    ps = ppool.tile([M, GW], f32, name=f"ps_{q}_{j}", tag=f"ps{j}")
                    for c in range(NCHUNK):
                        nc.tensor.matmul(
                            ps[:, :],
                            A_sb[:, c, :],
                            x_sb[:, b, c, :],
                            start=(c == 0),
                            stop=(c == NCHUNK - 1),
                        )
                    # PSUM -> SBUF band write with per-partition bias add
                    nc.scalar.add(zT_sb[32 * j : 32 * j + 32, :], ps[:, :], d_sb[:])

                # 32x32 block transpose -> sample-major within each band
                Zq = zqpool.tile([128, GW], f32, name=f"Z_{q}", tag="Z")
                nc.vector.transpose(Zq[:], zT_sb[:])

                def emit_epilogue(q=q, Zq=Zq):
                    # softmax-weighted average, batched over the quad
                    Zb = Zq[:].rearrange("p (blk c) -> p blk c", c=32)
                    zt4 = Zb[:, :, 0:24].rearrange("p blk (t c) -> p blk t c", c=12)
                    lg = zt4[:, :, :, 0:6]
                    sc = zt4[:, :, :, 6:12]

                    ssum = epool.tile([128, NB, T], f32, name=f"ssum_{q}", tag="ssum")
                    num = epool.tile([128, NB, T], f32, name=f"num_{q}", tag="num")
                    rinv = epool.tile([128, NB, T], f32, name=f"rinv_{q}", tag="rinv")

                    nc.scalar.activation(lg, lg, mybir.ActivationFunctionType.Exp)
                    nc.vector.tensor_reduce(
                        ssum[:], lg, axis=mybir.AxisListType.X, op=mybir.AluOpType.add
                    )
                    nc.vector.tensor_mul(lg, lg, sc)  # exp * s, clobbers exp
                    nc.vector.tensor_reduce(
                        num[:], lg, axis=mybir.AxisListType.X, op=mybir.AluOpType.add
                    )
                    nc.vector.reciprocal(rinv[:], ssum[:])
                    nc.vector.tensor_mul(res_all[:, q, :, :], num[:], rinv[:])

                # defer one quad so epilogue ops don't head-of-line-block the
                # next quad's bias-adds (in-order engine queues)
                pending.append(emit_epilogue)
                if len(pending) > 1:
                    pending.popleft()()
            while pending:
                pending.popleft()()

            nc.sync.dma_start(out=out_ext[:, :, :, :], in_=res_all[:])

    nc.compile()
    return nc


_PROGRAM = None


def _ensure_ntff_hook():
    """Provide antenv.axon_hooks if the image lacks it (NTFF profiling)."""
    try:
        import antenv.axon_hooks  # noqa: F401

        return
    except ImportError:
        pass
    import contextlib
    import ctypes
    import sys
    import types

    import antenv

    mod = types.ModuleType("antenv.axon_hooks")
    holder = {"hook": None}
    mod.set_axon_ntff_profile_hook = lambda h: holder.__setitem__("hook", h)
    mod.get_axon_ntff_profile_hook = lambda: holder["hook"]
    sys.modules["antenv.axon_hooks"] = mod
    antenv.axon_hooks = mod

    so_path = "/opt/axon/libaxon_pjrt.so"
    try:
        lib = ctypes.CDLL(so_path)
    except OSError:
        return
    if not hasattr(lib, "axon_start_nrt_profile"):
        return
    lib.axon_start_nrt_profile.argtypes = [
        ctypes.POINTER(ctypes.c_int64),
        ctypes.c_size_t,
    ]
    lib.axon_start_nrt_profile.restype = ctypes.c_int64
    lib.axon_stop_nrt_profile.argtypes = [ctypes.c_char_p]
    lib.axon_stop_nrt_profile.restype = ctypes.c_int64

    @contextlib.contextmanager
    def _hook(output_dir, device_ids):
        import jax

        jax.devices()
        if device_ids:
            ids = (ctypes.c_int64 * len(device_ids))(*device_ids)
            rc = lib.axon_start_nrt_profile(ids, len(device_ids))
        else:
            rc = lib.axon_start_nrt_profile(None, 0)
        if rc != 0:
            raise RuntimeError(f"axon_start_nrt_profile rc={rc}")
        try:
            yield
        finally:
            n = lib.axon_stop_nrt_profile(str(output_dir).encode())
            print(f"ntff profile: {n} file(s) written to {output_dir}")

    mod.set_axon_ntff_profile_hook(_hook)


def _run(inputs, trace=False):
    global _PROGRAM
    import ml_dtypes

    import concourse.bass_utils as bass_utils

    if trace:
        _ensure_ntff_hook()
        # keep trace artifacts local; no bucket in this sandbox
        bass_utils.upload_artifacts = lambda tmpdir: "local://" + tmpdir

    A, d = _fold(inputs)
    A16 = A.astype(ml_dtypes.bfloat16)
    x = np.asarray(inputs["x"], np.float32)
    in_maps = []
    for i in range(N_CORES):
        # [I, BS] -> [c, p, g, w] -> [g, p, c, w], bf16 contiguous
        xT = x[i * BS : (i + 1) * BS].T.reshape(NCHUNK, 128, 4 * NQ, GW)
        xq = np.ascontiguousarray(xT.transpose(2, 1, 0, 3)).astype(ml_dtypes.bfloat16)
        in_maps.append({"xq": xq, "A": A16, "d": d})

    if _PROGRAM is None:
        _PROGRAM = _build_program()

    kres = bass_utils.run_bass_kernel_spmd(
        _PROGRAM, in_maps, core_ids=list(range(N_CORES)), trace=trace
    )

    parts = []
    for i in range(N_CORES):
        o = np.asarray(kres.results[i]["out"])  # [128, NQ, NB, T]
        # s = q*QW + j*GW + 32*blk + r with p = 32*j + r
        o = o.reshape(4, 32, NQ, NB, T)  # j, r, q, blk, t
        parts.append(o.transpose(4, 2, 0, 3, 1).reshape(T, BS))
    full = np.concatenate(parts, axis=1)[:, :, None].astype(np.float32)
    return full, kres


def kernel(**inputs):
    out, _ = _run(inputs, trace=bool(int(os.environ.get("KERNEL_TRACE", "0"))))
    return out


# revision 13
# speedup vs baseline: 1.0721x; 1.0128x over previous
"""Trainium2 Bass kernel for the CGC multi-task MoE routing module.

Math: the reference computes, per task t:
    expert outputs  E[t,e] = x @ W[t,e] + b[t,e]          (ES specific + EC common)
    gate logits     L[t]   = concat_e(E[t,e]) @ Wg[t] + bg[t]
    weights         p      = softmax(L[t])
    feature         F[t]   = sum_e p_e * E[t,e]
    out[t]          = F[t] @ Wt[t] + bt[t]                # scalar per sample

Both L[t] and the per-expert scalars s[t,e] = E[t,e] @ Wt[t] are linear in x,
so everything folds into one skinny matmul z = x @ A + d with
A: [I, 24] (per task: 6 logit cols + 6 scalar cols), followed by a per-sample
6-way softmax-weighted average:
    out[t,b] = sum_e exp(L_e) * s_e / sum_e exp(L_e)      (bt folded into s).

The kernel is memory-bound on streaming x. Device traffic is halved by
shipping x (and A) as bfloat16 — quantization rel-err ~1.7e-3, well inside
the 2e-2 gate — with fp32 PSUM accumulation and an fp32 epilogue.

Device kernel (SPMD over 8 cores, batch-sharded):
  - host packs each x shard as bf16 [NQ, 128, NCHUNK, QW]: per (quad q,
    partition p) a contiguous 16 KB run covering all 4 I-chunks, so each
    quad loads with 128 large descriptors.
  - all quad loads are issued up front (8.4 MB bf16 fits SBUF easily).
  - per quad: 4 bands x 4 accumulating bf16 matmuls (lhsT = A chunk
    [128, 32], rhs = x [128, 512]) -> zT [32, GW] in PSUM; ScalarE copies
    PSUM->SBUF adding the per-partition bias d; VectorE 32x32
    block-transpose flips to a per-sample layout.
  - epilogue per quad: Exp on ScalarE, group-reductions, multiply and
    reciprocal on VectorE, results accumulated in one SBUF tile.
  - single DMA of the [128, NQ*NB*T] result tile at the end.
"""

import os

import numpy as np

B, I, H = 65536, 512, 128
T, ES, EC = 2, 2, 4
ETOT = ES + EC

N_CORES = 8
BS = B // N_CORES  # samples per core
M = 32  # folded output channels, padded 24 -> 32 for the 32x32 transpose
GW = 512  # samples per band (one PSUM bank per band)
QW = 4 * GW  # samples per quad (4 bands stacked on the 128 partitions)
NQ = BS // QW
NCHUNK = I // 128
NB = GW // 32  # 32-sample blocks per band


def _fold(inputs):
    """Fold all weights into A [I, M] and bias d [M, 1] (float32).

    Channel layout per task t (base 12*t): 0:6 gate logits, 6:12 per-expert
    scalars (bt folded in, valid since softmax weights sum to 1).
    """
    w64 = lambda k: np.asarray(inputs[k], np.float64)
    Wc, bc, Ws, bs = w64("Wc"), w64("bc"), w64("Ws"), w64("bs")
    Wg, bg, Wt, bt = w64("Wg"), w64("bg"), w64("Wt"), w64("bt")

    A = np.zeros((I, M))
    d = np.zeros(M)
    for t in range(T):
        W_all = np.concatenate(
            [Ws[t, e] for e in range(ES)] + [Wc[e] for e in range(EC)], axis=1
        )  # [I, ETOT*H]
        b_all = np.concatenate(
            [bs[t, e] for e in range(ES)] + [bc[e] for e in range(EC)]
        )  # [ETOT*H]
        A[:, 12 * t : 12 * t + 6] = W_all @ Wg[t]
        d[12 * t : 12 * t + 6] = b_all @ Wg[t] + bg[t]
        A[:, 12 * t + 6 : 12 * t + 12] = (
            W_all.reshape(I, ETOT, H) * Wt[t, :, 0][None, None, :]
        ).sum(-1)
        d[12 * t + 6 : 12 * t + 12] = (
            b_all.reshape(ETOT, H) * Wt[t, :, 0][None, :]
        ).sum(-1) + bt[t, 0]
    return A.astype(np.float32), d.reshape(M, 1).astype(np.float32)


def _build_program():
    import concourse.bacc as bacc
    import concourse.mybir as mybir
    from concourse.tile import TileContext

    f32 = mybir.dt.float32
    bf16 = mybir.dt.bfloat16

    nc = bacc.Bacc("TRN2", target_bir_lowering=False, debug=False, num_devices=N_CORES)
    # xq[g, p, c, w]: I-index = c*128 + p, sample = g*GW + w  (g = band)
    xq_ext = nc.declare_dram_parameter("xq", [4 * NQ, 128, NCHUNK, GW], bf16, isOutput=False)
    # host pre-packs A into SBUF layout [p, c*m] so the load is a plain 2D copy
    A_ext = nc.declare_dram_parameter("A", [128, NCHUNK * M], bf16, isOutput=False)
    d_ext = nc.declare_dram_parameter("d", [M, 1], f32, isOutput=False)
    # out[p, q, blk, t]: sample s = q*QW + (p//32)*GW + 32*blk + p%32, task t
    out_ext = nc.declare_dram_parameter("out", [128, NQ, NB, T], f32, isOutput=True)

    with TileContext(nc) as tc:
        with (
            tc.tile_pool(name="consts", bufs=1) as cpool,
            tc.tile_pool(name="xin", bufs=1) as xpool,
            tc.tile_pool(name="zt", bufs=3) as ztpool,
            tc.tile_pool(name="zq", bufs=3) as zqpool,
            tc.tile_pool(name="epi", bufs=4) as epool,
            tc.tile_pool(name="psum", bufs=2, space="PSUM") as ppool,
        ):
            # prefetch all of x up front (8.4 MB, fits SBUF).  Load sizes are
            # graded: single bands at the start so the first matmul fires
            # ~0.7us into the stream, big blocks in the middle (each
            # dma_start costs ~0.6us serial descriptor-gen on Sync plus
            # fixed semaphore overhead), single bands at the end so the
            # final quad's compute isn't stuck behind a 2MB transfer.
            # Band 0 is issued before even A/d so data flows ASAP.
            load_bands = [1, 1, 2, 4, 4, 2, 1, 1]
            x_tiles = []  # per band g: (tile, band offset within tile)
            x_pending = []

            def issue_load(li, g0, nb):
                x_sb = xpool.tile(
                    [128, nb, NCHUNK, GW], bf16, name=f"x_{li}", tag=f"x{li}"
                )
                nc.sync.dma_start(
                    out=x_sb[:],
                    in_=xq_ext[g0 : g0 + nb, :, :, :].rearrange("g p c w -> p g c w"),
                )
                for b in range(nb):
                    x_tiles.append((x_sb, b))

            issue_load(0, 0, load_bands[0])

            A_sb = cpool.tile([128, NCHUNK, M], bf16)
            nc.sync.dma_start(
                out=A_sb[:].rearrange("p c m -> p (c m)"), in_=A_ext[:, :]
            )
            d_sb = cpool.tile([M, 1], f32)
            nc.sync.dma_start(out=d_sb[:], in_=d_ext[:, :])

            g0 = load_bands[0]
            for li, nb in enumerate(load_bands[1:], start=1):
                issue_load(li, g0, nb)
                g0 += nb

            from collections import deque

            pending = deque()
            for q in range(NQ):
                # z^T for the 4 bands, stacked on partitions by the bias-add
                zT_sb = ztpool.tile([128, GW], f32, name=f"zTsb_{q}", tag="zTsb")
                for j in range(4):
                    x_sb, b = x_tiles[4 * q + j]
                    ps = ppool.tile([M, GW], f32, name=f"ps_{q}_{j}", tag=f"ps{j}")
                    for c in range(NCHUNK):
                        nc.tensor.matmul(
                            ps[:, :],
                            A_sb[:, c, :],
                            x_sb[:, b, c, :],
                            start=(c == 0),
                            stop=(c == NCHUNK - 1),
                        )
                    # PSUM -> SBUF band write with per-partition bias add
                    nc.scalar.add(zT_sb[32 * j : 32 * j + 32, :], ps[:, :], d_sb[:])

                # last quad: transpose/epilogue/store in 2 half-width pieces
                # so the tail after the final matmul is shorter
                pieces = [(0, GW)] if q < NQ - 1 else [(0, GW // 2), (GW // 2, GW // 2)]

                def emit_piece(q, zT_sb, off, w, pi):
                    # 32x32 block transpose -> sample-major within each band
                    Zq = zqpool.tile([128, w], f32, name=f"Z_{q}_{pi}", tag=f"Z{pi}")
                    nc.vector.transpose(Zq[:], zT_sb[:, off : off + w])

                    def emit_epilogue():
                        # softmax-weighted average over the piece
                        nb = w // 32
                        b0 = off // 32
                        Zb = Zq[:].rearrange("p (blk c) -> p blk c", c=32)
                        zt4 = Zb[:, :, 0:24].rearrange("p blk (t c) -> p blk t c", c=12)
                        lg = zt4[:, :, :, 0:6]
                        sc = zt4[:, :, :, 6:12]

                        ssum = epool.tile([128, nb, T], f32, name=f"ssum_{q}_{pi}", tag=f"ssum{pi}")
                        num = epool.tile([128, nb, T], f32, name=f"num_{q}_{pi}", tag=f"num{pi}")
                        rinv = epool.tile([128, nb, T], f32, name=f"rinv_{q}_{pi}", tag=f"rinv{pi}")
                        res = epool.tile([128, nb, T], f32, name=f"res_{q}_{pi}", tag=f"res{pi}")

                        nc.scalar.activation(lg, lg, mybir.ActivationFunctionType.Exp)
                        nc.vector.tensor_reduce(
                            ssum[:], lg, axis=mybir.AxisListType.X, op=mybir.AluOpType.add
                        )
                        nc.vector.tensor_mul(lg, lg, sc)  # exp * s, clobbers exp
                        nc.vector.tensor_reduce(
                            num[:], lg, axis=mybir.AxisListType.X, op=mybir.AluOpType.add
                        )
                        nc.vector.reciprocal(rinv[:], ssum[:])
                        nc.vector.tensor_mul(res[:], num[:], rinv[:])
                        nc.sync.dma_start(
                            out=out_ext[:, q, b0 : b0 + nb, :], in_=res[:]
                        )

                    return emit_epilogue

                for pi, (off, w) in enumerate(pieces):
                    # defer one piece so epilogue ops don't head-of-line-block
                    # the next quad's bias-adds (in-order engine queues)
                    pending.append(emit_piece(q, zT_sb, off, w, pi))
                    if len(pending) > 1:
                        pending.popleft()()
            while pending:
                pending.popleft()()

    nc.compile()
    return nc


_PROGRAM = None


def _ensure_ntff_hook():
    """Provide antenv.axon_hooks if the image lacks it (NTFF profiling)."""
    try:
        import antenv.axon_hooks  # noqa: F401

        return
    except ImportError:
        pass
    import contextlib
    import ctypes
    import sys
    import types

    import antenv

    mod = types.ModuleType("antenv.axon_hooks")
    holder = {"hook": None}
    mod.set_axon_ntff_profile_hook = lambda h: holder.__setitem__("hook", h)
    mod.get_axon_ntff_profile_hook = lambda: holder["hook"]
    sys.modules["antenv.axon_hooks"] = mod
    antenv.axon_hooks = mod

    so_path = "/opt/axon/libaxon_pjrt.so"
    try:
        lib = ctypes.CDLL(so_path)
    except OSError:
        return
    if not hasattr(lib, "axon_start_nrt_profile"):
        return
    lib.axon_start_nrt_profile.argtypes = [
        ctypes.POINTER(ctypes.c_int64),
        ctypes.c_size_t,
    ]
    lib.axon_start_nrt_profile.restype = ctypes.c_int64
    lib.axon_stop_nrt_profile.argtypes = [ctypes.c_char_p]
    lib.axon_stop_nrt_profile.restype = ctypes.c_int64

    @contextlib.contextmanager
    def _hook(output_dir, device_ids):
        import jax

        jax.devices()
        if device_ids:
            ids = (ctypes.c_int64 * len(device_ids))(*device_ids)
            rc = lib.axon_start_nrt_profile(ids, len(device_ids))
        else:
            rc = lib.axon_start_nrt_profile(None, 0)
        if rc != 0:
            raise RuntimeError(f"axon_start_nrt_profile rc={rc}")
        try:
            yield
        finally:
            n = lib.axon_stop_nrt_profile(str(output_dir).encode())
            print(f"ntff profile: {n} file(s) written to {output_dir}")

    mod.set_axon_ntff_profile_hook(_hook)


def _run(inputs, trace=False):
    global _PROGRAM
    import ml_dtypes

    import concourse.bass_utils as bass_utils

    if trace:
        _ensure_ntff_hook()
        # keep trace artifacts local; no bucket in this sandbox
        bass_utils.upload_artifacts = lambda tmpdir: "local://" + tmpdir

    A, d = _fold(inputs)
    # pack A into the SBUF lhsT layout [p, c*M + m], I-index = c*128 + p
    A16 = np.ascontiguousarray(
        A.reshape(NCHUNK, 128, M).transpose(1, 0, 2).reshape(128, NCHUNK * M)
    ).astype(ml_dtypes.bfloat16)
    x = np.asarray(inputs["x"], np.float32)
    in_maps = []
    for i in range(N_CORES):
        # [I, BS] -> [c, p, g, w] -> [g, p, c, w], bf16 contiguous
        xT = x[i * BS : (i + 1) * BS].T.reshape(NCHUNK, 128, 4 * NQ, GW)
        xq = np.ascontiguousarray(xT.transpose(2, 1, 0, 3)).astype(ml_dtypes.bfloat16)
        in_maps.append({"xq": xq, "A": A16, "d": d})

    if _PROGRAM is None:
        _PROGRAM = _build_program()

    kres = bass_utils.run_bass_kernel_spmd(
        _PROGRAM, in_maps, core_ids=list(range(N_CORES)), trace=trace
    )

    parts = []
    for i in range(N_CORES):
        o = np.asarray(kres.results[i]["out"])  # [128, NQ, NB, T]
        # s = q*QW + j*GW + 32*blk + r with p = 32*j + r
        o = o.reshape(4, 32, NQ, NB, T)  # j, r, q, blk, t
        parts.append(o.transpose(4, 2, 0, 3, 1).reshape(T, BS))
    full = np.concatenate(parts, axis=1)[:, :, None].astype(np.float32)
    return full, kres


def kernel(**inputs):
    out, _ = _run(inputs, trace=bool(int(os.environ.get("KERNEL_TRACE", "0"))))
    return out


# revision 17
# speedup vs baseline: 1.0910x; 1.0176x over previous
"""Trainium2 Bass kernel for the CGC multi-task MoE routing module.

Math: the reference computes, per task t:
    expert outputs  E[t,e] = x @ W[t,e] + b[t,e]          (ES specific + EC common)
    gate logits     L[t]   = concat_e(E[t,e]) @ Wg[t] + bg[t]
    weights         p      = softmax(L[t])
    feature         F[t]   = sum_e p_e * E[t,e]
    out[t]          = F[t] @ Wt[t] + bt[t]                # scalar per sample

Both L[t] and the per-expert scalars s[t,e] = E[t,e] @ Wt[t] are linear in x,
so everything folds into one skinny matmul z = x @ A + d with
A: [I, 24] (per task: 6 logit cols + 6 scalar cols), followed by a per-sample
6-way softmax-weighted average:
    out[t,b] = sum_e exp(L_e) * s_e / sum_e exp(L_e)      (bt folded into s).

The kernel is memory-bound on streaming x. Device traffic is halved by
shipping x (and A) as bfloat16 — quantization rel-err ~1.7e-3, well inside
the 2e-2 gate — with fp32 PSUM accumulation and an fp32 epilogue.

Device kernel (SPMD over 8 cores, batch-sharded):
  - host packs each x shard as bf16 [NQ, 128, NCHUNK, QW]: per (quad q,
    partition p) a contiguous 16 KB run covering all 4 I-chunks, so each
    quad loads with 128 large descriptors.
  - all quad loads are issued up front (8.4 MB bf16 fits SBUF easily).
  - per quad: 4 bands x 4 accumulating bf16 matmuls (lhsT = A chunk
    [128, 32], rhs = x [128, 512]) -> zT [32, GW] in PSUM; ScalarE copies
    PSUM->SBUF adding the per-partition bias d; VectorE 32x32
    block-transpose flips to a per-sample layout.
  - epilogue per quad: Exp on ScalarE, group-reductions, multiply and
    reciprocal on VectorE, results accumulated in one SBUF tile.
  - single DMA of the [128, NQ*NB*T] result tile at the end.
"""

import os

import numpy as np

B, I, H = 65536, 512, 128
T, ES, EC = 2, 2, 4
ETOT = ES + EC

N_CORES = 8
BS = B // N_CORES  # samples per core
M = 32  # folded output channels, padded 24 -> 32 for the 32x32 transpose
GW = 512  # samples per band (one PSUM bank per band)
QW = 4 * GW  # samples per quad (4 bands stacked on the 128 partitions)
NQ = BS // QW
NCHUNK = I // 128
NB = GW // 32  # 32-sample blocks per band


def _fold(inputs):
    """Fold all weights into A [I, M] and bias d [M, 1] (float32).

    Channel layout per task t (base 12*t): 0:6 gate logits, 6:12 per-expert
    scalars (bt folded in, valid since softmax weights sum to 1).
    """
    w64 = lambda k: np.asarray(inputs[k], np.float64)
    Wc, bc, Ws, bs = w64("Wc"), w64("bc"), w64("Ws"), w64("bs")
    Wg, bg, Wt, bt = w64("Wg"), w64("bg"), w64("Wt"), w64("bt")

    A = np.zeros((I, M))
    d = np.zeros(M)
    for t in range(T):
        W_all = np.concatenate(
            [Ws[t, e] for e in range(ES)] + [Wc[e] for e in range(EC)], axis=1
        )  # [I, ETOT*H]
        b_all = np.concatenate(
            [bs[t, e] for e in range(ES)] + [bc[e] for e in range(EC)]
        )  # [ETOT*H]
        A[:, 12 * t : 12 * t + 6] = W_all @ Wg[t]
        d[12 * t : 12 * t + 6] = b_all @ Wg[t] + bg[t]
        A[:, 12 * t + 6 : 12 * t + 12] = (
            W_all.reshape(I, ETOT, H) * Wt[t, :, 0][None, None, :]
        ).sum(-1)
        d[12 * t + 6 : 12 * t + 12] = (
            b_all.reshape(ETOT, H) * Wt[t, :, 0][None, :]
        ).sum(-1) + bt[t, 0]
    return A.astype(np.float32), d.reshape(M, 1).astype(np.float32)


def _build_program():
    import concourse.bacc as bacc
    import concourse.mybir as mybir
    from concourse.tile import TileContext

    f32 = mybir.dt.float32
    bf16 = mybir.dt.bfloat16

    nc = bacc.Bacc("TRN2", target_bir_lowering=False, debug=False, num_devices=N_CORES)
    # xq[g, p, c, w]: I-index = c*128 + p, sample = g*GW + w  (g = band)
    xq_ext = nc.declare_dram_parameter("xq", [4 * NQ, 128, NCHUNK, GW], bf16, isOutput=False)
    # host pre-packs A into SBUF layout [p, c*m] so the load is a plain 2D copy
    A_ext = nc.declare_dram_parameter("A", [128, NCHUNK * M], bf16, isOutput=False)
    d_ext = nc.declare_dram_parameter("d", [M, 1], f32, isOutput=False)
    # out[p, q, blk, t]: sample s = q*QW + (p//32)*GW + 32*blk + p%32, task t
    out_ext = nc.declare_dram_parameter("out", [128, NQ, NB, T], f32, isOutput=True)

    with TileContext(nc) as tc:
        with (
            tc.tile_pool(name="consts", bufs=1) as cpool,
            tc.tile_pool(name="xin", bufs=1) as xpool,
            tc.tile_pool(name="zt", bufs=3) as ztpool,
            tc.tile_pool(name="zq", bufs=3) as zqpool,
            tc.tile_pool(name="epi", bufs=4) as epool,
            tc.tile_pool(name="psum", bufs=2, space="PSUM") as ppool,
        ):
            # prefetch all of x up front (8.4 MB, fits SBUF).  Load sizes are
            # graded: single bands at the start so the first matmul fires
            # ~0.7us into the stream, big blocks in the middle (each
            # dma_start costs ~0.6us serial descriptor-gen on Sync plus
            # fixed semaphore overhead), single bands at the end so the
            # final quad's compute isn't stuck behind a 2MB transfer.
            # Band 0 is issued before even A/d so data flows ASAP.
            load_bands = [1, 1, 2, 4, 4, 2, 1, 1]
            x_tiles = []  # per band g: (tile, band offset within tile)
            x_pending = []

            def issue_load(li, g0, nb):
                x_sb = xpool.tile(
                    [128, nb, NCHUNK, GW], bf16, name=f"x_{li}", tag=f"x{li}"
                )
                nc.sync.dma_start(
                    out=x_sb[:],
                    in_=xq_ext[g0 : g0 + nb, :, :, :].rearrange("g p c w -> p g c w"),
                )
                for b in range(nb):
                    x_tiles.append((x_sb, b))

            issue_load(0, 0, load_bands[0])

            A_sb = cpool.tile([128, NCHUNK, M], bf16)
            nc.sync.dma_start(
                out=A_sb[:].rearrange("p c m -> p (c m)"), in_=A_ext[:, :]
            )
            d_sb = cpool.tile([M, 1], f32)
            nc.sync.dma_start(out=d_sb[:], in_=d_ext[:, :])

            g0 = load_bands[0]
            for li, nb in enumerate(load_bands[1:], start=1):
                issue_load(li, g0, nb)
                g0 += nb

            # PE p-state warmup: the PE ramps to full clock only after ~3us
            # of continuous work (cold matmuls run 2-2.7x slower).  Spin it
            # on scratch data while the first x bands are still in flight so
            # the real matmuls all run at full rate.
            warm = cpool.tile([128, GW], bf16)
            nc.gpsimd.memset(warm[:], 0.0)
            ps_w = ppool.tile([M, GW], f32, name="ps_warm", tag="ps0")
            for _ in range(12):
                nc.tensor.matmul(
                    ps_w[:, :], warm[:, 0:M], warm[:, :], start=True, stop=True
                )

            from collections import deque

            pending = deque()
            for q in range(NQ):
                # z^T for the 4 bands, stacked on partitions by the bias-add
                zT_sb = ztpool.tile([128, GW], f32, name=f"zTsb_{q}", tag="zTsb")
                last_band = q == NQ - 1
                for j in range(4):
                    x_sb, b = x_tiles[4 * q + j]
                    ps = ppool.tile([M, GW], f32, name=f"ps_{q}_{j}", tag=f"ps{j}")
                    for c in range(NCHUNK):
                        nc.tensor.matmul(
                            ps[:, :],
                            A_sb[:, c, :],
                            x_sb[:, b, c, :],
                            start=(c == 0),
                            stop=(c == NCHUNK - 1),
                        )
                    # PSUM -> SBUF band write with per-partition bias add;
                    # final band in halves so the first transpose piece can
                    # start sooner
                    if last_band and j == 3:
                        h = GW // 2
                        nc.scalar.add(
                            zT_sb[32 * j : 32 * j + 32, 0:h], ps[:, 0:h], d_sb[:]
                        )
                        nc.scalar.add(
                            zT_sb[32 * j : 32 * j + 32, h:GW], ps[:, h:GW], d_sb[:]
                        )
                    else:
                        nc.scalar.add(zT_sb[32 * j : 32 * j + 32, :], ps[:, :], d_sb[:])

                # last quad: transpose/epilogue in 2 half-width pieces so the
                # tail after the final matmul is shorter (single store)
                pieces = [(0, GW)] if q < NQ - 1 else [(0, GW // 2), (GW // 2, GW // 2)]
                res_q = epool.tile([128, NB, T], f32, name=f"res_{q}", tag="res")

                def emit_piece(q, zT_sb, res_q, off, w, pi, store):
                    # 32x32 block transpose -> sample-major within each band
                    Zq = zqpool.tile([128, w], f32, name=f"Z_{q}_{pi}", tag=f"Z{pi}")
                    nc.vector.transpose(Zq[:], zT_sb[:, off : off + w])

                    def emit_epilogue():
                        # softmax-weighted average over the piece
                        nb = w // 32
                        b0 = off // 32
                        Zb = Zq[:].rearrange("p (blk c) -> p blk c", c=32)
                        zt4 = Zb[:, :, 0:24].rearrange("p blk (t c) -> p blk t c", c=12)
                        lg = zt4[:, :, :, 0:6]
                        sc = zt4[:, :, :, 6:12]

                        ssum = epool.tile([128, nb, T], f32, name=f"ssum_{q}_{pi}", tag=f"ssum{pi}")
                        num = epool.tile([128, nb, T], f32, name=f"num_{q}_{pi}", tag=f"num{pi}")
                        rinv = epool.tile([128, nb, T], f32, name=f"rinv_{q}_{pi}", tag=f"rinv{pi}")

                        nc.scalar.activation(lg, lg, mybir.ActivationFunctionType.Exp)
                        nc.vector.tensor_reduce(
                            ssum[:], lg, axis=mybir.AxisListType.X, op=mybir.AluOpType.add
                        )
                        nc.vector.tensor_mul(lg, lg, sc)  # exp * s, clobbers exp
                        nc.vector.tensor_reduce(
                            num[:], lg, axis=mybir.AxisListType.X, op=mybir.AluOpType.add
                        )
                        nc.vector.reciprocal(rinv[:], ssum[:])
                        nc.vector.tensor_mul(res_q[:, b0 : b0 + nb, :], num[:], rinv[:])
                        if store:
                            nc.sync.dma_start(out=out_ext[:, q, :, :], in_=res_q[:])

                    return emit_epilogue

                for pi, (off, w) in enumerate(pieces):
                    # defer one piece so epilogue ops don't head-of-line-block
                    # the next quad's bias-adds (in-order engine queues)
                    store = pi == len(pieces) - 1
                    pending.append(emit_piece(q, zT_sb, res_q, off, w, pi, store))
                    if len(pending) > 1:
                        pending.popleft()()
            while pending:
                pending.popleft()()

    nc.compile()
    return nc


_PROGRAM = None


def _ensure_ntff_hook():
    """Provide antenv.axon_hooks if the image lacks it (NTFF profiling)."""
    try:
        import antenv.axon_hooks  # noqa: F401

        return
    except ImportError:
        pass
    import contextlib
    import ctypes
    import sys
    import types

    import antenv

    mod = types.ModuleType("antenv.axon_hooks")
    holder = {"hook": None}
    mod.set_axon_ntff_profile_hook = lambda h: holder.__setitem__("hook", h)
    mod.get_axon_ntff_profile_hook = lambda: holder["hook"]
    sys.modules["antenv.axon_hooks"] = mod
    antenv.axon_hooks = mod

    so_path = "/opt/axon/libaxon_pjrt.so"
    try:
        lib = ctypes.CDLL(so_path)
    except OSError:
        return
    if not hasattr(lib, "axon_start_nrt_profile"):
        return
    lib.axon_start_nrt_profile.argtypes = [
        ctypes.POINTER(ctypes.c_int64),
        ctypes.c_size_t,
    ]
    lib.axon_start_nrt_profile.restype = ctypes.c_int64
    lib.axon_stop_nrt_profile.argtypes = [ctypes.c_char_p]
    lib.axon_stop_nrt_profile.restype = ctypes.c_int64

    @contextlib.contextmanager
    def _hook(output_dir, device_ids):
        import jax

        jax.devices()
        if device_ids:
            ids = (ctypes.c_int64 * len(device_ids))(*device_ids)
            rc = lib.axon_start_nrt_profile(ids, len(device_ids))
        else:
            rc = lib.axon_start_nrt_profile(None, 0)
        if rc != 0:
            raise RuntimeError(f"axon_start_nrt_profile rc={rc}")
        try:
            yield
        finally:
            n = lib.axon_stop_nrt_profile(str(output_dir).encode())
            print(f"ntff profile: {n} file(s) written to {output_dir}")

    mod.set_axon_ntff_profile_hook(_hook)


def _run(inputs, trace=False):
    global _PROGRAM
    import ml_dtypes

    import concourse.bass_utils as bass_utils

    if trace:
        _ensure_ntff_hook()
        # keep trace artifacts local; no bucket in this sandbox
        bass_utils.upload_artifacts = lambda tmpdir: "local://" + tmpdir

    A, d = _fold(inputs)
    # pack A into the SBUF lhsT layout [p, c*M + m], I-index = c*128 + p
    A16 = np.ascontiguousarray(
        A.reshape(NCHUNK, 128, M).transpose(1, 0, 2).reshape(128, NCHUNK * M)
    ).astype(ml_dtypes.bfloat16)
    x = np.asarray(inputs["x"], np.float32)
    in_maps = []
    for i in range(N_CORES):
        # [I, BS] -> [c, p, g, w] -> [g, p, c, w], bf16 contiguous
        xT = x[i * BS : (i + 1) * BS].T.reshape(NCHUNK, 128, 4 * NQ, GW)
        xq = np.ascontiguousarray(xT.transpose(2, 1, 0, 3)).astype(ml_dtypes.bfloat16)
        in_maps.append({"xq": xq, "A": A16, "d": d})

    if _PROGRAM is None:
        _PROGRAM = _build_program()

    kres = bass_utils.run_bass_kernel_spmd(
        _PROGRAM, in_maps, core_ids=list(range(N_CORES)), trace=trace
    )

    parts = []
    for i in range(N_CORES):
        o = np.asarray(kres.results[i]["out"])  # [128, NQ, NB, T]
        # s = q*QW + j*GW + 32*blk + r with p = 32*j + r
        o = o.reshape(4, 32, NQ, NB, T)  # j, r, q, blk, t
        parts.append(o.transpose(4, 2, 0, 3, 1).reshape(T, BS))
    full = np.concatenate(parts, axis=1)[:, :, None].astype(np.float32)
    return full, kres


def kernel(**inputs):
    out, _ = _run(inputs, trace=bool(int(os.environ.get("KERNEL_TRACE", "0"))))
    return out


# revision 22
# speedup vs baseline: 1.1281x; 1.0340x over previous
"""Trainium2 Bass kernel for the CGC multi-task MoE routing module.

Math: the reference computes, per task t:
    expert outputs  E[t,e] = x @ W[t,e] + b[t,e]          (ES specific + EC common)
    gate logits     L[t]   = concat_e(E[t,e]) @ Wg[t] + bg[t]
    weights         p      = softmax(L[t])
    feature         F[t]   = sum_e p_e * E[t,e]
    out[t]          = F[t] @ Wt[t] + bt[t]                # scalar per sample

Both L[t] and the per-expert scalars s[t,e] = E[t,e] @ Wt[t] are linear in x,
so everything folds into one skinny matmul z = x @ A + d with
A: [I, 24] (per task: 6 logit cols + 6 scalar cols), followed by a per-sample
6-way softmax-weighted average:
    out[t,b] = sum_e exp(L_e) * s_e / sum_e exp(L_e)      (bt folded into s).

The kernel is memory-bound on streaming x. Device traffic is halved by
shipping x (and A) as bfloat16 — quantization rel-err ~1.7e-3, well inside
the 2e-2 gate — with fp32 PSUM accumulation and an fp32 epilogue.

Device kernel (SPMD over 8 cores, batch-sharded):
  - host packs each x shard as bf16 [NQ, 128, NCHUNK, QW]: per (quad q,
    partition p) a contiguous 16 KB run covering all 4 I-chunks, so each
    quad loads with 128 large descriptors.
  - all quad loads are issued up front (8.4 MB bf16 fits SBUF easily).
  - per quad: 4 bands x 4 accumulating bf16 matmuls (lhsT = A chunk
    [128, 32], rhs = x [128, 512]) -> zT [32, GW] in PSUM; ScalarE copies
    PSUM->SBUF adding the per-partition bias d; VectorE 32x32
    block-transpose flips to a per-sample layout.
  - epilogue per quad: Exp on ScalarE, group-reductions, multiply and
    reciprocal on VectorE, results accumulated in one SBUF tile.
  - single DMA of the [128, NQ*NB*T] result tile at the end.
"""

import os

import numpy as np

B, I, H = 65536, 512, 128
T, ES, EC = 2, 2, 4
ETOT = ES + EC

N_CORES = 8
BS = B // N_CORES  # samples per core
M = 32  # folded output channels, padded 24 -> 32 for the 32x32 transpose
GW = 512  # samples per band (one PSUM bank per band)
QW = 4 * GW  # samples per quad (4 bands stacked on the 128 partitions)
NQ = BS // QW
NCHUNK = I // 128
NB = GW // 32  # 32-sample blocks per band


def _fold(inputs):
    """Fold all weights into A [I, M] and bias d [M, 1] (float32).

    Channel layout per task t (base 12*t): 0:6 gate logits, 6:12 per-expert
    scalars (bt folded in, valid since softmax weights sum to 1).
    """
    w64 = lambda k: np.asarray(inputs[k], np.float64)
    Wc, bc, Ws, bs = w64("Wc"), w64("bc"), w64("Ws"), w64("bs")
    Wg, bg, Wt, bt = w64("Wg"), w64("bg"), w64("Wt"), w64("bt")

    A = np.zeros((I, M))
    d = np.zeros(M)
    for t in range(T):
        W_all = np.concatenate(
            [Ws[t, e] for e in range(ES)] + [Wc[e] for e in range(EC)], axis=1
        )  # [I, ETOT*H]
        b_all = np.concatenate(
            [bs[t, e] for e in range(ES)] + [bc[e] for e in range(EC)]
        )  # [ETOT*H]
        A[:, 12 * t : 12 * t + 6] = W_all @ Wg[t]
        d[12 * t : 12 * t + 6] = b_all @ Wg[t] + bg[t]
        A[:, 12 * t + 6 : 12 * t + 12] = (
            W_all.reshape(I, ETOT, H) * Wt[t, :, 0][None, None, :]
        ).sum(-1)
        d[12 * t + 6 : 12 * t + 12] = (
            b_all.reshape(ETOT, H) * Wt[t, :, 0][None, :]
        ).sum(-1) + bt[t, 0]
    return A.astype(np.float32), d.reshape(M, 1).astype(np.float32)


def _build_program():
    import concourse.bacc as bacc
    import concourse.mybir as mybir
    from concourse.tile import TileContext

    f32 = mybir.dt.float32
    bf16 = mybir.dt.bfloat16

    nc = bacc.Bacc("TRN2", target_bir_lowering=False, debug=False, num_devices=N_CORES)
    # xq[g, p, c, w]: I-index = c*128 + p, sample = g*GW + w  (g = band)
    xq_ext = nc.declare_dram_parameter("xq", [4 * NQ, 128, NCHUNK, GW], bf16, isOutput=False)
    # host pre-packs A into SBUF layout [p, c*m] so the load is a plain 2D copy
    A_ext = nc.declare_dram_parameter("A", [128, NCHUNK * M], bf16, isOutput=False)
    d_ext = nc.declare_dram_parameter("d", [M, 1], f32, isOutput=False)
    # out[p, q, blk, t]: sample s = q*QW + (p//32)*GW + 32*blk + p%32, task t
    out_ext = nc.declare_dram_parameter("out", [128, NQ, NB, T], f32, isOutput=True)

    with TileContext(nc) as tc:
        with (
            tc.tile_pool(name="consts", bufs=1) as cpool,
            tc.tile_pool(name="xin", bufs=1) as xpool,
            tc.tile_pool(name="zt", bufs=3) as ztpool,
            tc.tile_pool(name="zq", bufs=3) as zqpool,
            tc.tile_pool(name="epi", bufs=4) as epool,
            tc.tile_pool(name="psum", bufs=2, space="PSUM") as ppool,
        ):
            # prefetch all of x up front (8.4 MB, fits SBUF).  Load sizes are
            # graded: single bands at the start so the first matmul fires
            # ~0.7us into the stream, big blocks in the middle (each
            # dma_start costs ~0.6us serial descriptor-gen on Sync plus
            # fixed semaphore overhead), single bands at the end so the
            # final quad's compute isn't stuck behind a 2MB transfer.
            # Band 0 is issued before even A/d so data flows ASAP.
            load_bands = [1, 1, 2, 2, 2, 2, 2, 2, 1, 1]
            x_tiles = []  # per band g: (tile, band offset within tile)
            x_pending = []

            def issue_load(li, g0, nb):
                x_sb = xpool.tile(
                    [128, nb, NCHUNK, GW], bf16, name=f"x_{li}", tag=f"x{li}"
                )
                nc.sync.dma_start(
                    out=x_sb[:],
                    in_=xq_ext[g0 : g0 + nb, :, :, :].rearrange("g p c w -> p g c w"),
                )
                for b in range(nb):
                    x_tiles.append((x_sb, b))

            issue_load(0, 0, load_bands[0])

            A_sb = cpool.tile([128, NCHUNK, M], bf16)
            nc.sync.dma_start(
                out=A_sb[:].rearrange("p c m -> p (c m)"), in_=A_ext[:, :]
            )
            d_sb = cpool.tile([M, 1], f32)
            nc.sync.dma_start(out=d_sb[:], in_=d_ext[:, :])

            g0 = load_bands[0]
            for li, nb in enumerate(load_bands[1:], start=1):
                issue_load(li, g0, nb)
                g0 += nb

            # PE p-state warmup: the PE ramps to full clock only after ~3us
            # of continuous work (cold matmuls run 2-2.7x slower).  Spin it
            # on scratch data while the first x bands are still in flight so
            # the real matmuls all run at full rate.
            warm = cpool.tile([128, GW], bf16)
            nc.gpsimd.memset(warm[:], 0.0)
            ps_w = ppool.tile([M, GW], f32, name="ps_warm", tag="ps0")
            for _ in range(7):
                nc.tensor.matmul(
                    ps_w[:, :], warm[:, 0:M], warm[:, :], start=True, stop=True
                )

            from collections import deque

            pending = deque()
            for q in range(NQ):
                # z^T for the 4 bands, stacked on partitions by the bias-add
                zT_sb = ztpool.tile([128, GW], f32, name=f"zTsb_{q}", tag="zTsb")
                last_band = q == NQ - 1
                for j in range(4):
                    x_sb, b = x_tiles[4 * q + j]
                    ps = ppool.tile([M, GW], f32, name=f"ps_{q}_{j}", tag=f"ps{j}")
                    for c in range(NCHUNK):
                        nc.tensor.matmul(
                            ps[:, :],
                            A_sb[:, c, :],
                            x_sb[:, b, c, :],
                            start=(c == 0),
                            stop=(c == NCHUNK - 1),
                        )
                    # PSUM -> SBUF band write with per-partition bias add;
                    # final band in halves so the first transpose piece can
                    # start sooner
                    if last_band and j == 3:
                        h = GW // 2
                        nc.scalar.add(
                            zT_sb[32 * j : 32 * j + 32, 0:h], ps[:, 0:h], d_sb[:]
                        )
                        nc.scalar.add(
                            zT_sb[32 * j : 32 * j + 32, h:GW], ps[:, h:GW], d_sb[:]
                        )
                    else:
                        nc.scalar.add(zT_sb[32 * j : 32 * j + 32, :], ps[:, :], d_sb[:])

                # last quad: transpose/epilogue in 2 half-width pieces so the
                # tail after the final matmul is shorter (single store)
                pieces = [(0, GW)] if q < NQ - 1 else [(0, GW // 2), (GW // 2, GW // 2)]
                res_q = epool.tile([128, NB, T], f32, name=f"res_{q}", tag="res")

                def emit_piece(q, zT_sb, res_q, off, w, pi, store, on_gpsimd=False):
                    # 32x32 block transpose -> sample-major within each band
                    Zq = zqpool.tile([128, w], f32, name=f"Z_{q}_{pi}", tag=f"Z{pi}")
                    nc.vector.transpose(Zq[:], zT_sb[:, off : off + w])

                    def emit_epilogue():
                        # softmax-weighted average over the piece
                        nb = w // 32
                        b0 = off // 32
                        Zb = Zq[:].rearrange("p (blk c) -> p blk c", c=32)
                        zt4 = Zb[:, :, 0:24].rearrange("p blk (t c) -> p blk t c", c=12)
                        lg = zt4[:, :, :, 0:6]
                        sc = zt4[:, :, :, 6:12]

                        ssum = epool.tile([128, nb, T], f32, name=f"ssum_{q}_{pi}", tag=f"ssum{pi}")
                        num = epool.tile([128, nb, T], f32, name=f"num_{q}_{pi}", tag=f"num{pi}")

                        nc.scalar.activation(lg, lg, mybir.ActivationFunctionType.Exp)
                        nc.vector.tensor_reduce(
                            ssum[:], lg, axis=mybir.AxisListType.X, op=mybir.AluOpType.add
                        )
                        nc.vector.tensor_mul(lg, lg, sc)  # exp * s, clobbers exp
                        nc.vector.tensor_reduce(
                            num[:], lg, axis=mybir.AxisListType.X, op=mybir.AluOpType.add
                        )
                        rinv = epool.tile(
                            [128, nb, T], f32, name=f"rinv_{q}_{pi}", tag=f"rinv{pi}"
                        )
                        nc.vector.reciprocal(rinv[:], ssum[:])
                        nc.vector.tensor_mul(
                            res_q[:, b0 : b0 + nb, :], num[:], rinv[:]
                        )
                        if store:
                            nc.sync.dma_start(out=out_ext[:, q, :, :], in_=res_q[:])

                    return emit_epilogue

                for pi, (off, w) in enumerate(pieces):
                    # defer one piece so epilogue ops don't head-of-line-block
                    # the next quad's bias-adds (in-order engine queues)
                    store = pi == len(pieces) - 1
                    on_gpsimd = q == NQ - 1 and pi == 1
                    pending.append(
                        emit_piece(q, zT_sb, res_q, off, w, pi, store, on_gpsimd)
                    )
                    if len(pending) > 1:
                        pending.popleft()()
            while pending:
                pending.popleft()()

    nc.compile()
    return nc


_PROGRAM = None


def _ensure_ntff_hook():
    """Provide antenv.axon_hooks if the image lacks it (NTFF profiling)."""
    try:
        import antenv.axon_hooks  # noqa: F401

        return
    except ImportError:
        pass
    import contextlib
    import ctypes
    import sys
    import types

    import antenv

    mod = types.ModuleType("antenv.axon_hooks")
    holder = {"hook": None}
    mod.set_axon_ntff_profile_hook = lambda h: holder.__setitem__("hook", h)
    mod.get_axon_ntff_profile_hook = lambda: holder["hook"]
    sys.modules["antenv.axon_hooks"] = mod
    antenv.axon_hooks = mod

    so_path = "/opt/axon/libaxon_pjrt.so"
    try:
        lib = ctypes.CDLL(so_path)
    except OSError:
        return
    if not hasattr(lib, "axon_start_nrt_profile"):
        return
    lib.axon_start_nrt_profile.argtypes = [
        ctypes.POINTER(ctypes.c_int64),
        ctypes.c_size_t,
    ]
    lib.axon_start_nrt_profile.restype = ctypes.c_int64
    lib.axon_stop_nrt_profile.argtypes = [ctypes.c_char_p]
    lib.axon_stop_nrt_profile.restype = ctypes.c_int64

    @contextlib.contextmanager
    def _hook(output_dir, device_ids):
        import jax

        jax.devices()
        if device_ids:
            ids = (ctypes.c_int64 * len(device_ids))(*device_ids)
            rc = lib.axon_start_nrt_profile(ids, len(device_ids))
        else:
            rc = lib.axon_start_nrt_profile(None, 0)
        if rc != 0:
            raise RuntimeError(f"axon_start_nrt_profile rc={rc}")
        try:
            yield
        finally:
            n = lib.axon_stop_nrt_profile(str(output_dir).encode())
            print(f"ntff profile: {n} file(s) written to {output_dir}")

    mod.set_axon_ntff_profile_hook(_hook)


def _run(inputs, trace=False):
    global _PROGRAM
    import ml_dtypes

    import concourse.bass_utils as bass_utils

    if trace:
        _ensure_ntff_hook()
        # keep trace artifacts local; no bucket in this sandbox
        bass_utils.upload_artifacts = lambda tmpdir: "local://" + tmpdir

    A, d = _fold(inputs)
    # pack A into the SBUF lhsT layout [p, c*M + m], I-index = c*128 + p
    A16 = np.ascontiguousarray(
        A.reshape(NCHUNK, 128, M).transpose(1, 0, 2).reshape(128, NCHUNK * M)
    ).astype(ml_dtypes.bfloat16)
    x = np.asarray(inputs["x"], np.float32)
    in_maps = []
    for i in range(N_CORES):
        # [I, BS] -> [c, p, g, w] -> [g, p, c, w], bf16 contiguous
        xT = x[i * BS : (i + 1) * BS].T.reshape(NCHUNK, 128, 4 * NQ, GW)
        xq = np.ascontiguousarray(xT.transpose(2, 1, 0, 3)).astype(ml_dtypes.bfloat16)
        in_maps.append({"xq": xq, "A": A16, "d": d})

    if _PROGRAM is None:
        _PROGRAM = _build_program()

    kres = bass_utils.run_bass_kernel_spmd(
        _PROGRAM, in_maps, core_ids=list(range(N_CORES)), trace=trace
    )

    parts = []
    for i in range(N_CORES):
        o = np.asarray(kres.results[i]["out"])  # [128, NQ, NB, T]
        # s = q*QW + j*GW + 32*blk + r with p = 32*j + r
        o = o.reshape(4, 32, NQ, NB, T)  # j, r, q, blk, t
        parts.append(o.transpose(4, 2, 0, 3, 1).reshape(T, BS))
    full = np.concatenate(parts, axis=1)[:, :, None].astype(np.float32)
    return full, kres


def kernel(**inputs):
    out, _ = _run(inputs, trace=bool(int(os.environ.get("KERNEL_TRACE", "0"))))
    return out
